# revision 39
# baseline (speedup 1.0000x reference)
"""Trainium2 Bass kernel for nn_DenseFilterExpansion.

Computes out[b, f, t] = x[b, 0, t] * w[f, t] + bias[f, t] for
x: (128, 1, 4096), w/bias: (256, 4096)  ->  out: (128, 256, 4096) fp32.

Strategy (per core, data-parallel over batch, 16 batches/core):
  - The kernel is HBM-write-bound, so the device computes and stores the
    output in bf16 (half the write bytes of fp32); the host widens the
    result to fp32. End-to-end relative error ~2e-3 (x, w, and the
    product each rounded once to bf16).
  - x arrives as a (16, 4096) bf16 block, resident in SBUF. Per batch, a
    K=1 ones-matmul on TensorE broadcasts the row across 128 partitions
    into PSUM (fp32), and ScalarE (ACT) cast-copies PSUM -> SBUF bf16.
  - w stays resident in SBUF as two (128, 4096) bf16 tiles. VectorE
    multiplies (tensor_tensor, all-bf16 SBUF operands -> 2x perf mode,
    ~2.3 us per (batch, f-chunk) tile; 73 us total vs ~95 us of DMA).
  - Each (batch, f-chunk) bf16 tile is stored with one 1 MiB HWDGE DMA,
    alternating the SP and ACT rings.
Per-core HBM traffic: 32 MiB out + ~2.2 MiB in, vs 68 MiB for the fp32
variant (which measures ~185 us at the ~360 GB/s per-core limit).
"""

import numpy as np
import ml_dtypes

import concourse.bacc as bacc
import concourse.bass as bass
import concourse.mybir as mybir
import concourse.tile as tile
from concourse.bass_utils import run_bass_kernel_spmd

N_CORES = 8
B_FULL = 128
F = 256
T = 4096
BS = B_FULL // N_CORES  # batches per core = 16
P = 128                 # partitions
FP = F // P             # f-chunks = 2
TH = 1024               # psum tile width (2 banks)
MM_N = 512              # matmul free dim (one PSUM bank, ISA cap)
NH = T // TH            # 2 psum halves per batch

_nc_cache: dict = {}


def _build(with_bias: bool) -> bass.Bass:
    f32 = mybir.dt.float32
    bf16 = mybir.dt.bfloat16
    nc = bacc.Bacc("TRN2", debug=False)

    f8 = mybir.dt.float8e4
    # x ships as an exact-ish Dekker pair hi+lo in fp8-e4m3 (combined
    # representation error ~8e-4, better than one bf16 rounding). The
    # DoubleRow matmul sums the pair while broadcasting, at 2x PE row
    # rate.
    x_d = nc.dram_tensor("x2", [BS, 2 * T], f8, kind="ExternalInput")
    sel_d = nc.dram_tensor("sel2", [BS, BS * 2 * P], f8, kind="ExternalInput")
    w_d = nc.dram_tensor("w", [F, T], bf16, kind="ExternalInput")
    b_d = (
        nc.dram_tensor("bvec", [F, T], bf16, kind="ExternalInput")
        if with_bias
        else None
    )
    o_d = nc.dram_tensor("out", [BS, F, T], bf16, kind="ExternalOutput")

    with tile.TileContext(nc) as tc:
        with (
            tc.tile_pool(name="const", bufs=1) as cpool,
            tc.tile_pool(name="xbp", bufs=5) as xpool,
            tc.tile_pool(name="outp", bufs=8) as opool,
            tc.tile_pool(name="psum", bufs=4, space="PSUM") as ppool,
        ):
            # Selection matrix (host-built): sel[k, (bi, r, p)] = (k ==
            # bi). A DoubleRow K=16x2 fp8 matmul with lhsT = sel[:, bi]
            # broadcasts (and sums) the x hi/lo pair of row bi across
            # the 128 output partitions, reading the resident x block at
            # base partition 0 (HW requires matmul operands at base
            # partition 0/32/64). This keeps all mid-kernel DMA off
            # SWDGE (whose descriptor-ring traffic makes SDMA engine 15
            # a straggler).
            sel = cpool.tile([BS, BS * 2 * P], f8, tag="sel")
            nc.sync.dma_start(out=sel[:], in_=sel_d[:, :])

            # x hi/lo block resident on partitions 0-15 (one 128 KiB
            # HWDGE DMA). sel + x2 go first on the SP ring so the matmul
            # pipeline can start as early as possible.
            x_sb = cpool.tile([BS, 2 * T], f8, tag="x2")
            nc.sync.dma_start(out=x_sb[:], in_=x_d[:, :])
            x_rt = x_sb[0:BS, :].rearrange("k (r t) -> k r t", r=2)

            w_sb = {}
            b_sb = {}
            for c in range(FP):
                wt = cpool.tile([P, T], bf16, tag=f"w{c}", name=f"w{c}")
                # Both w tiles on the ACT ring: keeps them off the DMA
                # semaphore lane the first matmuls wait on (sel2+x2 on
                # SP), so PE starts ~4 us earlier.
                nc.scalar.dma_start(out=wt[:], in_=w_d[c * P : (c + 1) * P, :])
                w_sb[c] = wt
                if with_bias:
                    bt = cpool.tile([P, T], bf16, tag=f"b{c}", name=f"b{c}")
                    nc.gpsimd.dma_start(
                        out=bt[:], in_=b_d[c * P : (c + 1) * P, :]
                    )
                    b_sb[c] = bt

            for bi in range(BS):
                # Broadcast x row bi across 128 partitions: selection
                # matmul into PSUM (fp32), then ACT cast-copies to bf16
                # SBUF.
                xb = xpool.tile([P, T], bf16, tag="xb", name=f"xb{bi}")
                for h in range(NH):
                    ps = ppool.tile([P, TH], f32, tag="ps", name=f"ps{bi}_{h}")
                    for j in range(TH // MM_N):
                        col = h * TH + j * MM_N
                        nc.tensor.matmul(
                            ps[:, j * MM_N : (j + 1) * MM_N],
                            sel[0:BS, bi * 2 * P : (bi + 1) * 2 * P].rearrange(
                                "k (r p) -> k r p", r=2
                            ),
                            x_rt[:, :, col : col + MM_N],
                            start=True,
                            stop=True,
                            perf_mode=mybir.MatmulPerfMode.DoubleRow,
                        )
                    nc.scalar.copy(
                        out=xb[:, h * TH : (h + 1) * TH], in_=ps[:]
                    )
                for c in range(FP):
                    ot = opool.tile([P, T], bf16, tag="ot", name=f"ot{bi}_{c}")
                    # all-bf16 SBUF tensor_tensor -> DVE 2x perf mode
                    nc.vector.tensor_mul(out=ot[:], in0=w_sb[c][:], in1=xb[:])
                    if with_bias:
                        nc.vector.tensor_add(
                            out=ot[:], in0=ot[:], in1=b_sb[c][:]
                        )
                    # Alternate stores across both HWDGE rings.
                    ring = nc.sync if (bi * FP + c) % 2 == 0 else nc.scalar
                    ring.dma_start(
                        out=o_d[bi, c * P : (c + 1) * P, :], in_=ot[:]
                    )
    nc.finalize()
    return nc


def _get_nc(with_bias: bool) -> bass.Bass:
    if with_bias not in _nc_cache:
        _nc_cache[with_bias] = _build(with_bias)
    return _nc_cache[with_bias]


def _prepare(inputs: np.ndarray, w: np.ndarray, b: np.ndarray):
    """Host-side prep shared by kernel() and the traced test path."""
    bf = ml_dtypes.bfloat16
    f8 = ml_dtypes.float8_e4m3
    x = np.ascontiguousarray(inputs.reshape(B_FULL, T), dtype=np.float32)
    with_bias = bool(np.any(b))
    wb = np.ascontiguousarray(w).astype(bf)
    bb = np.ascontiguousarray(b).astype(bf) if with_bias else None

    # Exact-ish fp8 Dekker pair: hi + lo == x to ~8e-4 (fp32 sum).
    hi = x.astype(f8)
    lo = (x - hi.astype(np.float32)).astype(f8)
    x2 = np.stack([hi, lo], axis=1).reshape(B_FULL, 2 * T)

    sel = np.zeros((BS, BS, 2, P), dtype=f8)
    for bi in range(BS):
        sel[bi, bi, :, :] = 1.0
    sel = sel.reshape(BS, BS * 2 * P)

    nc = _get_nc(with_bias)
    in_maps = []
    for c in range(N_CORES):
        m = {
            "x2": np.ascontiguousarray(x2[c * BS : (c + 1) * BS]),
            "sel2": sel,
            "w": wb,
        }
        if with_bias:
            m["bvec"] = bb
        in_maps.append(m)
    return nc, in_maps


def _finish(res) -> np.ndarray:
    out = np.concatenate([np.asarray(r["out"]) for r in res.results], axis=0)
    return out.astype(np.float32)


def kernel(inputs: np.ndarray, w: np.ndarray, b: np.ndarray, **kw) -> np.ndarray:
    nc, in_maps = _prepare(inputs, w, b)
    res = run_bass_kernel_spmd(nc, in_maps, core_ids=list(range(N_CORES)))
    return _finish(res)
